# revision 10
# baseline (speedup 1.0000x reference)
"""Trainium2 Bass kernel for nn_Correspondence (retrieval_knn).

Pipeline per clip (B=4 clips, snip=8 frames of 28x28, C=256):
  xs = [C, THW=6272] per clip; corr = cosine similarity over channels;
  per column j: top-5 rows t (same-frame block excluded) -> gather xs cols,
  max over the 5 -> y; global BatchNorm (training stats) + relu -> 1x1 conv
  -> + identity.

Sharding: 8 cores = 4 clips x 2 column-halves. Each core gets its clip's
xs ROTATED by half the frames so its own j-range is local columns [0,3136)
— the same SPMD program runs on all cores. Same-frame masking is handled
by *never computing* the own-frame columns (frame-rotated chunk order).

Precision: the correlation matmul runs in float32r (12-bit mantissa,
full PE rate); top-5 ranking from those values gives final rel err
~5e-3 on the real data distribution (gate 2e-2). Gather/BN run in
exact fp32; the 1x1 conv uses single f32r.
"""
import sys, os
import numpy as np

for _p in ("/opt/trn_rl_repo", "/root/.axon_site/_ro/trn_rl_repo"):
    if os.path.isdir(_p) and _p not in sys.path:
        sys.path.insert(0, _p)
        break

import ml_dtypes

# ---------------- problem constants (hardcoded) ----------------
C = 256          # channels
SNIP = 8         # frames per clip
F = 784          # 28*28
T = SNIP * F     # 6272 columns per clip
J = T // 2       # 3136 columns handled per core
JT = 112         # j-tile rows (one PE M-tile; 112*7 = 784 -> tiles never span frames)
NT = J // JT     # 28 j-tiles
TS = 7 * F       # 5488 searched columns per j-tile (own frame excluded)
HALF = TS // 2   # 2744
QUART = TS // 4  # 1372
KTOP = 5
NCORES = 8
NTOT = 32 * F    # batchnorm count = BS*H*W = 25088
CW = 392         # matmul chunk width (one PSUM bank)
GAT = JT * 8     # 896 gather indices per j-tile (top-5 + 3 duplicated slots)

_CACHE = {}


def _round_f32r(x):
    """Round-to-nearest-even to f32r (low 12 mantissa bits zeroed)."""
    b = np.ascontiguousarray(x, np.float32).view(np.uint32)
    low = b & np.uint32(0xFFF)
    add = (low > 0x800) | ((low == 0x800) & (((b >> 12) & 1) == 1))
    b = (b & ~np.uint32(0xFFF)) + (add.astype(np.uint32) << 12)
    return b.view(np.float32)


def _build(num_cores, dbg=False, sim=False):
    import concourse.bass as bass
    import concourse.mybir as mybir
    import concourse.tile as tile
    from concourse import bacc
    from concourse.masks import make_identity

    fp32 = mybir.dt.float32
    f32r = mybir.dt.float32r
    bf16 = mybir.dt.bfloat16
    i16 = mybir.dt.int16
    u16 = mybir.dt.uint16
    Alu = mybir.AluOpType
    Act = mybir.ActivationFunctionType
    Ax = mybir.AxisListType

    nc = bacc.Bacc("TRN2", target_bir_lowering=False, debug=False,
                   num_devices=num_cores)

    xs_d = nc.declare_dram_parameter("xs", [C, T], fp32, isOutput=False)
    r_d = nc.declare_dram_parameter("xr", [C, T], f32r, isOutput=False)
    wt_d = nc.declare_dram_parameter("wt", [C, C], f32r, isOutput=False)
    gam_d = nc.declare_dram_parameter("gam", [C], fp32, isOutput=False)
    bet_d = nc.declare_dram_parameter("bet", [C], fp32, isOutput=False)
    cb_d = nc.declare_dram_parameter("cb", [C], fp32, isOutput=False)
    out_d = nc.declare_dram_parameter("out", [4, C, F], fp32, isOutput=True)
    if dbg:
        dbg_sA = nc.declare_dram_parameter("dbg_sA", [JT, HALF], fp32, isOutput=True)
        dbg_sB = nc.declare_dram_parameter("dbg_sB", [JT, HALF], fp32, isOutput=True)
        dbg_t8 = nc.declare_dram_parameter("dbg_t8", [JT, 8], fp32, isOutput=True)
        dbg_ia = nc.declare_dram_parameter("dbg_ia", [JT, 4, 8], mybir.dt.uint16, isOutput=True)
        dbg_fin = nc.declare_dram_parameter("dbg_fin", [JT, 8], fp32, isOutput=True)
        dbg_w16 = nc.declare_dram_parameter("dbg_w16", [128, GAT // 16], i16, isOutput=True)
        dbg_gat = nc.declare_dram_parameter("dbg_gat", [128, GAT], fp32, isOutput=True)
        dbg_yt = nc.declare_dram_parameter("dbg_yt", [128, JT], fp32, isOutput=True)

    with tile.TileContext(nc) as tc:
        with tc.tile_pool(name="singles", bufs=1) as sg, \
             tc.tile_pool(name="dram", bufs=1, space="DRAM") as dp:
            # ---- persistent inputs in SBUF
            xs0 = sg.tile([128, T], fp32)
            xs1 = sg.tile([128, T], fp32)
            r0 = sg.tile([128, T], f32r)
            r1 = sg.tile([128, T], f32r)
            wt0 = sg.tile([128, C], f32r)
            wt1 = sg.tile([128, C], f32r)
            gam = sg.tile([128, 2], fp32)
            bet = sg.tile([128, 2], fp32)
            cbv = sg.tile([128, 2], fp32)
            ident = sg.tile([128, 128], fp32)
            bases = sg.tile([112, 4, 8], fp32)
            stats = sg.tile([128, 4, NT // 4], fp32)
            astat = sg.tile([128, 4], fp32)
            scales = sg.tile([128, 2], fp32)
            shifts = sg.tile([128, 2], fp32)
            scr = sg.tile([128, 4 * JT], fp32)

            y_dram = dp.tile([2, 128, J], fp32)
            cc_in = dp.tile([128, 4], fp32)
            cc_out = dp.tile([128, 4], fp32, addr_space="Shared")

            nc.sync.dma_start(out=r0, in_=r_d[0:128, :])
            nc.sync.dma_start(out=r1, in_=r_d[128:256, :])
            nc.sync.dma_start(out=xs0, in_=xs_d[0:128, :])
            nc.sync.dma_start(out=xs1, in_=xs_d[128:256, :])
            nc.sync.dma_start(out=wt0, in_=wt_d[0:128, :])
            nc.sync.dma_start(out=wt1, in_=wt_d[128:256, :])
            nc.sync.dma_start(out=gam[:, 0:1], in_=gam_d[0:128])
            nc.sync.dma_start(out=gam[:, 1:2], in_=gam_d[128:256])
            nc.sync.dma_start(out=bet[:, 0:1], in_=bet_d[0:128])
            nc.sync.dma_start(out=bet[:, 1:2], in_=bet_d[128:256])
            nc.sync.dma_start(out=cbv[:, 0:1], in_=cb_d[0:128])
            nc.sync.dma_start(out=cbv[:, 1:2], in_=cb_d[128:256])

            make_identity(nc, ident)
            for k in range(4):
                nc.vector.memset(bases[:, k, :], float(k * QUART))

            xs_t = (xs0, xs1)

            with tc.tile_pool(name="spool", bufs=2) as sp, \
                 tc.tile_pool(name="work", bufs=2) as wk, \
                 tc.tile_pool(name="gidx", bufs=2) as gw, \
                 tc.tile_pool(name="gatp", bufs=1) as gp, \
                 tc.tile_pool(name="pp", bufs=6, space="PSUM") as pp, \
                 tc.tile_pool(name="pt", bufs=2, space="PSUM") as pt:

                for jt in range(NT):
                    f = jt // 7                      # local frame of this j-tile
                    j0 = jt * JT
                    sA = sp.tile([JT, HALF], fp32, tag="sA")
                    sB = sp.tile([JT, HALF], fp32, tag="sB")

                    # ---- correlation matmuls, frame-rotated column order
                    # chunks of 392 (= half frame): one PSUM bank each.
                    # f32r-only (no bf16 cross-correction): measured final
                    # rel err 5.1e-3 on the real data distribution, well
                    # inside the 2e-2 gate, and 3x fewer PE cycles.
                    for ci in range(14):
                        g = (f + 1 + ci // 2) % SNIP  # source frame for chunk
                        gc = g * F + (ci % 2) * CW
                        ps = pp.tile([JT, CW], fp32, tag="ps")
                        nc.tensor.matmul(ps, r0[:, j0:j0 + JT],
                                         r0[:, gc:gc + CW], start=True, stop=False)
                        nc.tensor.matmul(ps, r1[:, j0:j0 + JT],
                                         r1[:, gc:gc + CW], start=False, stop=True)
                        # drain PSUM -> s (ACT engine); 2744 = 7*392 exactly
                        if ci < 7:
                            nc.scalar.copy(sA[:, ci * CW:(ci + 1) * CW], ps[:])
                        else:
                            nc.scalar.copy(sB[:, (ci - 7) * CW:(ci - 6) * CW],
                                           ps[:])

                    if dbg and jt == 0:
                        nc.sync.dma_start(out=dbg_sA[:], in_=sA)
                        nc.sync.dma_start(out=dbg_sB[:], in_=sB)
                    # ---- top-8 values + indices (exact fp32)
                    t8ab = wk.tile([JT, 16], fp32, tag="t8ab")
                    t8 = wk.tile([JT, 8], fp32, tag="t8")
                    iall = wk.tile([JT, 4, 8], u16, tag="iall")
                    nc.vector.max(out=t8ab[:, 0:8], in_=sA)
                    nc.vector.max(out=t8ab[:, 8:16], in_=sB)
                    nc.vector.max(out=t8, in_=t8ab)
                    nc.vector.max_index(out=iall[:, 0, :], in_max=t8,
                                        in_values=sA[:, 0:QUART])
                    nc.vector.max_index(out=iall[:, 1, :], in_max=t8,
                                        in_values=sA[:, QUART:HALF])
                    nc.vector.max_index(out=iall[:, 2, :], in_max=t8,
                                        in_values=sB[:, 0:QUART])
                    nc.vector.max_index(out=iall[:, 3, :], in_max=t8,
                                        in_values=sB[:, QUART:HALF])

                    # ---- combine quarters -> global column index
                    fall = wk.tile([JT, 4, 8], fp32, tag="fall")
                    m01 = wk.tile([JT, 8], fp32, tag="m01")
                    m23 = wk.tile([JT, 8], fp32, tag="m23")
                    gmin = wk.tile([JT, 8], fp32, tag="gmin")
                    msk = wk.tile([JT, 8], fp32, tag="msk")
                    fin = wk.tile([JT, 8], fp32, tag="fin")
                    fdup = wk.tile([JT, 8], fp32, tag="fdup")
                    nc.vector.tensor_copy(fall, iall)          # u16 -> fp32 (65535 if absent)
                    nc.vector.tensor_add(fall, fall, bases)
                    nc.vector.tensor_tensor(out=m01, in0=fall[:, 0, :],
                                            in1=fall[:, 1, :], op=Alu.min)
                    nc.vector.tensor_tensor(out=m23, in0=fall[:, 2, :],
                                            in1=fall[:, 3, :], op=Alu.min)
                    nc.vector.tensor_tensor(out=gmin, in0=m01, in1=m23, op=Alu.min)
                    # searched col c -> clip col t = ((f+1)*784 + c) mod 6272
                    nc.vector.tensor_scalar_add(gmin, gmin, float((f + 1) * F))
                    nc.vector.tensor_scalar(out=msk, in0=gmin, scalar1=float(T),
                                            scalar2=None, op0=Alu.is_ge)
                    nc.vector.scalar_tensor_tensor(out=fin, in0=msk,
                                                   scalar=float(-T), in1=gmin,
                                                   op0=Alu.mult, op1=Alu.add)
                    nc.vector.tensor_copy(fdup[:, 0:5], fin[:, 0:5])
                    nc.vector.tensor_copy(fdup[:, 5:8],
                                          fin[:, 0:1].to_broadcast([JT, 3]))

                    if dbg and jt == 0:
                        nc.sync.dma_start(out=dbg_t8[:], in_=t8)
                        nc.sync.dma_start(out=dbg_ia[:], in_=iall)
                        nc.sync.dma_start(out=dbg_fin[:], in_=fin)
                    # ---- wrapped int16 index list, staged into a
                    # 4-tile group buffer (one ap_gather per 4 j-tiles:
                    # the gather cost is per-call ~ input width, so
                    # batching 4 tiles cuts Pool busy ~4x)
                    if jt % 4 == 0:
                        w16g = gw.tile([128, 4 * GAT // 16], i16, tag="w16g")
                    trp = pt.tile([8, JT], fp32, tag="tr")
                    nc.tensor.transpose(trp, fdup, ident[0:JT, 0:JT])
                    trs = wk.tile([8, JT], i16, tag="trs")
                    nc.vector.tensor_copy(trs, trp)
                    sl = w16g[:, (jt % 4) * (GAT // 16):(jt % 4 + 1) * (GAT // 16)]
                    trv = trs.rearrange("p (m two) -> p m two", two=2)
                    nc.sync.dma_start(out=sl[0:8, :], in_=trv[:, :, 0])
                    nc.sync.dma_start(out=sl[8:16, :], in_=trv[:, :, 1])
                    nc.sync.dma_start(out=sl[16:32, :], in_=sl[0:16, :])
                    nc.sync.dma_start(out=sl[32:64, :], in_=sl[0:32, :])
                    nc.sync.dma_start(out=sl[64:128, :], in_=sl[0:64, :])

                    # ---- gather + max over the 5 picks (+3 dups)
                    if jt % 4 == 3:
                        gi = jt // 4
                        jg0 = gi * 4 * JT
                        for c in range(2):
                            gat = gp.tile([128, 4 * GAT], fp32, tag=f"gat{c}")
                            nc.gpsimd.ap_gather(out_ap=gat[:], in_ap=xs_t[c][:],
                                                idxs_ap=w16g[:], channels=128,
                                                num_elems=T, d=1,
                                                num_idxs=4 * GAT)
                            yt = gp.tile([128, 4 * JT], fp32, tag=f"yt{c}")
                            gv = gat.rearrange("p (j k) -> p j k", k=8)
                            nc.vector.tensor_reduce(out=yt, in_=gv, axis=Ax.X,
                                                    op=Alu.max)
                            # batchnorm partial sums (ACT accumulator)
                            nc.scalar.activation(scr, yt, Act.Identity,
                                                 accum_out=stats[:, 2 * c, gi:gi + 1])
                            nc.scalar.activation(scr, yt, Act.Square,
                                                 accum_out=stats[:, 2 * c + 1, gi:gi + 1])
                            nc.sync.dma_start(out=y_dram[c, :, jg0:jg0 + 4 * JT],
                                              in_=yt)

            # ---- global batchnorm stats (allreduce over the 8 cores)
            nc.vector.tensor_reduce(out=astat, in_=stats, axis=Ax.X, op=Alu.add)
            nc.sync.dma_start(out=cc_in[:], in_=astat)
            if sim:
                # TimelineSim can't run collectives; model as a DRAM copy
                nc.sync.dma_start(out=cc_out[:], in_=cc_in[:])
            else:
                nc.gpsimd.collective_compute(
                    "AllReduce", Alu.add,
                    replica_groups=[list(range(num_cores))],
                    ins=[cc_in[:].opt()], outs=[cc_out[:].opt()])
            nc.sync.dma_start(out=astat, in_=cc_out[:])

            with tc.tile_pool(name="bnw", bufs=1) as bw:
                mean = bw.tile([128, 2], fp32)
                ex2 = bw.tile([128, 2], fp32)
                var = bw.tile([128, 2], fp32)
                std = bw.tile([128, 2], fp32)
                rstd = bw.tile([128, 2], fp32)
                vv = astat.rearrange("p (c m) -> p c m", m=2)
                nc.vector.tensor_scalar_mul(mean, vv[:, :, 0], 1.0 / NTOT)
                nc.vector.tensor_scalar_mul(ex2, vv[:, :, 1], 1.0 / NTOT)
                nc.vector.tensor_tensor(out=var, in0=mean, in1=mean, op=Alu.mult)
                nc.vector.tensor_sub(var, ex2, var)
                nc.vector.tensor_scalar_add(var, var, 1e-5)
                nc.scalar.sqrt(std, var)
                nc.vector.reciprocal(rstd, std)
                nc.vector.tensor_tensor(out=scales, in0=gam, in1=rstd, op=Alu.mult)
                nc.vector.tensor_tensor(out=shifts, in0=mean, in1=scales,
                                        op=Alu.mult)
                nc.vector.tensor_sub(shifts, bet, shifts)

            # ---- BN apply + relu + 1x1 conv + identity + store
            with tc.tile_pool(name="zp", bufs=2) as zp, \
                 tc.tile_pool(name="cp", bufs=2, space="PSUM") as cp:
                for ci in range(8):
                    c0 = ci * CW
                    yi0 = zp.tile([128, CW], fp32, tag="yi0")
                    yi1 = zp.tile([128, CW], fp32, tag="yi1")
                    nc.sync.dma_start(out=yi0, in_=y_dram[0, :, c0:c0 + CW])
                    nc.sync.dma_start(out=yi1, in_=y_dram[1, :, c0:c0 + CW])
                    z0 = zp.tile([128, CW], f32r, tag="z0")
                    z1 = zp.tile([128, CW], f32r, tag="z1")
                    nc.scalar.activation(z0, yi0, Act.Relu,
                                         bias=shifts[:, 0:1], scale=scales[:, 0:1])
                    nc.scalar.activation(z1, yi1, Act.Relu,
                                         bias=shifts[:, 1:2], scale=scales[:, 1:2])
                    fr, fc = divmod(ci, 2)
                    for ot in range(2):
                        o0 = ot * 128
                        cps = cp.tile([128, CW], fp32, tag="cps")
                        nc.tensor.matmul(cps, wt0[:, o0:o0 + 128], z0[:],
                                         start=True, stop=False)
                        nc.tensor.matmul(cps, wt1[:, o0:o0 + 128], z1[:],
                                         start=False, stop=True)
                        osb = zp.tile([128, CW], fp32, tag=f"osb{ot}")
                        nc.vector.scalar_tensor_tensor(
                            out=osb, in0=cps, scalar=cbv[:, ot:ot + 1],
                            in1=xs_t[ot][:, c0:c0 + CW], op0=Alu.add, op1=Alu.add)
                        nc.sync.dma_start(
                            out=out_d[fr, o0:o0 + 128, fc * CW:(fc + 1) * CW],
                            in_=osb)

    nc.finalize()
    return nc


def _get_nc(num_cores):
    if num_cores not in _CACHE:
        _CACHE[num_cores] = _build(num_cores)
    return _CACHE[num_cores]


def _prep_core_inputs(x, conv_w, gamma, beta, conv_b):
    """Build the 8 per-core input dicts from the full problem inputs."""
    xs_all = np.ascontiguousarray(
        x.reshape(4, SNIP, C, F).transpose(0, 2, 1, 3).reshape(4, C, T))
    wt = _round_f32r(np.ascontiguousarray(conv_w.T))
    maps = []
    for k in range(NCORES):
        b, h = divmod(k, 2)
        xs = xs_all[b]
        if h:
            xs = np.ascontiguousarray(
                np.concatenate((xs[:, J:], xs[:, :J]), axis=1))
        nrm = np.sqrt((xs * xs).sum(0, dtype=np.float32))
        xn = xs * (1.0 / nrm)[None, :].astype(np.float32)
        r = _round_f32r(xn)
        maps.append({
            "xs": xs,
            "xr": r,
            "wt": wt,
            "gam": np.ascontiguousarray(gamma, np.float32),
            "bet": np.ascontiguousarray(beta, np.float32),
            "cb": np.ascontiguousarray(conv_b, np.float32),
        })
    return maps


def kernel(x, gamma, beta, conv_w, conv_b, snip):
    assert int(snip) == SNIP and x.shape == (32, C, 28, 28)
    from concourse.bass_utils import run_bass_kernel_spmd

    x = np.ascontiguousarray(x, np.float32)
    maps = _prep_core_inputs(x, np.asarray(conv_w, np.float32),
                             np.asarray(gamma, np.float32),
                             np.asarray(beta, np.float32),
                             np.asarray(conv_b, np.float32))
    nc = _get_nc(NCORES)
    cores = list(range(NCORES))
    if not _CACHE.get("warm"):
        # First execution after comm setup once produced a corrupted
        # collective result; run once and discard, then run for real.
        run_bass_kernel_spmd(nc, maps, cores)
        _CACHE["warm"] = True
    res = run_bass_kernel_spmd(nc, maps, cores).results
    out = np.empty((32, C, F), np.float32)
    for k in range(NCORES):
        out[4 * k:4 * k + 4] = res[k]["out"]
    return out.reshape(32, C, 28, 28)

